# revision 6
# baseline (speedup 1.0000x reference)
"""Trainium2 Bass kernel for a 3-layer GCN (JKNet, mode='cat') — 8-core SPMD.

Strategy (dst-sharded graph parallelism, pipelined region AllGathers):
  - Nodes are partitioned across 8 cores (6250 each, padded to 6272 = 49*128),
    kept in IDENTITY order in the final accumulator and the gather table.
  - The per-layer table is split into 3 region tensors by slot range
    (wave1 = slots 32-48, wave2 = 16-31, wave3 = 0-15). Each wave's tail
    (bias+relu+transpose+matmul) finishes suffix-first and immediately
    AllGathers its region — so the collectives for layer L+1 overlap the
    remaining tail work of layer L and the early gathers of layer L+1.
  - Edges are split in two systems by src slot: sys B (src slot >= 32) reads
    region-1 rows (17408 < int16 range), sys A (src slot < 32) reads the
    combined region-2/3 tensor (exactly 32768 rows, max idx 32767). Each
    system degree-sorts its dst nodes so rounds are dense prefixes, making
    the accumulate a plain strided DVE add; both partial accumulators are
    reconciled into the identity-order accT through a DRAM bounce +
    permute-gather (foldB copies, foldA adds).
  - dma_gather runs on GPSIMD SWDGE (4 queues); descriptor generation is the
    kernel bottleneck, so CMAX=4096 amortizes per-instruction overhead.
  - bf16 is used for x, weights, transposes and matmul inputs (rel tol 2e-2).

Self-contained: hardcodes the problem geometry (N=50000, E=800000, 128->64,
3 layers, out 40) but computes all data-dependent schedules from the inputs.
"""

import sys

sys.path.insert(0, "/opt/trn_rl_repo")

import numpy as np

N = 50000
E = 800000
IN_DIM = 128
HID = 64
OUT_DIM = 40
M = 8               # cores
NPC = N // M        # 6250 nodes per core
SLOTS = 49          # ceil(6250/128)
SLICE = SLOTS * 128  # 6272 padded rows per core slice
CMAX = 4096         # max indices per dma_gather instruction
NQ = 4              # SWDGE queues

# waves, suffix-first: (slot_lo, slot_hi)
WAVES = [(32, SLOTS), (16, 32), (0, 16)]
R1_ROWS = M * 128 * (SLOTS - 32)   # 17408 (sys B table)
R23_ROWS = M * 128 * 32            # 32768 (sys A table; max idx 32767)


def _wrap16(a):
    """Flat [L] -> [128, L//16] int16, index j at partition j%16, slot j//16,
    replicated across the 8 GPSIMD core groups."""
    L = a.shape[0]
    return np.tile(a.reshape(L // 16, 16).T, (8, 1)).astype(np.int16)


def _wrap128(a):
    """Flat [L] -> [128, L//128], position j at partition j%128, slot j//128."""
    L = a.shape[0]
    return np.ascontiguousarray(a.reshape(L // 128, 128).T)


def _rowof(q):
    """acc position q -> wrapped DRAM row index (partition-major layout)."""
    return (q % 128) * SLOTS + q // 128


def _region_row(c, l):
    """Table row of node with local id l owned by core c, in its region
    tensor (sys B: region-1 rows; sys A: combined region-2/3 rows)."""
    m = l // 128
    p = l % 128
    return np.where(
        m >= 32, c * (128 * (SLOTS - 32)) + p * (SLOTS - 32) + (m - 32),
        np.where(m >= 16, c * 2048 + p * 16 + (m - 16),
                 16384 + c * 2048 + p * 16 + m))


def _ranks_within(p):
    """For int array p, rank of each element among equal values (stable)."""
    order = np.argsort(p, kind="stable")
    ps = p[order]
    starts = np.r_[0, np.nonzero(np.diff(ps))[0] + 1]
    counts = np.diff(np.r_[starts, len(ps)])
    r_sorted = np.arange(len(ps)) - np.repeat(starts, counts)
    r = np.empty_like(r_sorted)
    r[order] = r_sorted
    return r


def _plan_rounds(deg_by_core):
    """deg_by_core: [M, NPC] degree of each node (own sort order, desc).
    Returns list of global padded round widths (each a multiple of 128)."""
    smax = int(max(d[0] for d in deg_by_core)) if len(deg_by_core) else 0
    widths = []
    for s in range(smax):
        n_s = max(int((d > s).sum()) for d in deg_by_core)
        if n_s == 0:
            break
        widths.append(((n_s + 127) // 128) * 128)
    return widths


def _chunk_plan(widths):
    """Split flat [0, L) (concatenated padded rounds) into gather chunks of at
    most CMAX (multiples of 128), with per-chunk accumulate segments.
    Returns list of (off, w, [(msg_slot0, acc_slot0, nslots), ...])."""
    roundoff = np.r_[0, np.cumsum(widths)].astype(np.int64)
    L = int(roundoff[-1])
    chunks = []
    off = 0
    while off < L:
        w = min(CMAX, L - off)
        segs = []
        s = int(np.searchsorted(roundoff, off, side="right")) - 1
        a = off
        while a < off + w:
            b = min(off + w, int(roundoff[s + 1]))
            segs.append(((a - off) // 128, int(a - roundoff[s]) // 128, (b - a) // 128))
            a = b
            s += 1
        chunks.append((off, w, segs))
        off += w
    return chunks


def _build_system(pos_of_dst, row_of_src, ew, widths):
    """Flat (idx int64, ew f32) arrays for one round-system of one core."""
    roundoff = np.r_[0, np.cumsum(widths)]
    L = int(roundoff[-1])
    idx_flat = np.zeros(L, np.int64)
    ew_flat = np.zeros(L, np.float32)
    r = _ranks_within(pos_of_dst)
    flatpos = roundoff[r] + pos_of_dst
    idx_flat[flatpos] = row_of_src
    ew_flat[flatpos] = ew
    return idx_flat, ew_flat


def _prep(x, edge_index, edge_weight):
    src = np.asarray(edge_index[0], dtype=np.int64)
    dst = np.asarray(edge_index[1], dtype=np.int64)
    ew = np.asarray(edge_weight, dtype=np.float32)
    x = np.asarray(x, dtype=np.float32)

    dcore = dst // NPC
    dloc = dst - dcore * NPC
    score = src // NPC
    sloc = src - score * NPC
    isB = (sloc // 128) >= 32          # src in wave-1 slots -> sys B
    row = _region_row(score, sloc)     # region-relative table row of src

    posA_all, posB_all = [], []
    degA_sorted, degB_sorted = [], []
    for c in range(M):
        mask = dcore == c
        la = dloc[mask & ~isB]
        lb = dloc[mask & isB]
        degA = np.bincount(la, minlength=NPC)
        degB = np.bincount(lb, minlength=NPC)
        piA = np.argsort(-degA, kind="stable")
        piB = np.argsort(-degB, kind="stable")
        pA = np.empty(NPC, np.int64); pA[piA] = np.arange(NPC)
        pB = np.empty(NPC, np.int64); pB[piB] = np.arange(NPC)
        posA_all.append(pA)
        posB_all.append(pB)
        degA_sorted.append(degA[piA])
        degB_sorted.append(degB[piB])

    widthsA = _plan_rounds(degA_sorted)
    widthsB = _plan_rounds(degB_sorted)
    LA = int(np.sum(widthsA))
    LB = int(np.sum(widthsB))
    chunksA = _chunk_plan(widthsA)
    chunksB = _chunk_plan(widthsB)

    in_maps = []
    for c in range(M):
        mask = dcore == c
        mA = mask & ~isB
        mB = mask & isB
        idxA, ewA = _build_system(posA_all[c][dloc[mA]], row[mA], ew[mA], widthsA)
        idxB, ewB = _build_system(posB_all[c][dloc[mB]], row[mB], ew[mB], widthsB)
        assert idxA.max(initial=0) < R23_ROWS and idxB.max(initial=0) < R1_ROWS

        # fold maps: accT position q (identity) -> bounce row of the node's
        # position in the sorted accumulator. Pad positions map to pad rows
        # of the bounce (always zero).
        rhoA = _rowof(np.r_[posA_all[c], np.arange(NPC, SLICE)])
        rhoB = _rowof(np.r_[posB_all[c], np.arange(NPC, SLICE)])

        # x slice, transposed, identity order (pad columns zero)
        xT = np.zeros((IN_DIM, SLICE), np.float32)
        xT[:, :NPC] = x[c * NPC:(c + 1) * NPC, :].T

        in_maps.append({
            "xT": xT,
            "idxA": _wrap16(idxA), "ewA": _wrap128(ewA),
            "idxB": _wrap16(idxB), "ewB": _wrap128(ewB),
            "rhoA": _wrap16(rhoA), "rhoB": _wrap16(rhoB),
        })

    plan = {"LA": LA, "LB": LB, "chunksA": chunksA, "chunksB": chunksB}
    return plan, in_maps


def _build(plan, W1, b1, W2, b2, W3, b3, Wlin, blin):
    import concourse.bacc as bacc
    import concourse.mybir as mybir
    import concourse.tile as tile

    LA, LB = plan["LA"], plan["LB"]
    f32 = mybir.dt.float32
    bf16 = mybir.dt.bfloat16
    i16 = mybir.dt.int16

    nc = bacc.Bacc("TRN2", target_bir_lowering=False, debug=False,
                   num_devices=M, num_swdge_queues=NQ)

    # ---- I/O ----
    xT_d = nc.dram_tensor("xT", [IN_DIM, SLICE], bf16, kind="ExternalInput")
    idxA_d = nc.dram_tensor("idxA", [128, LA // 16], i16, kind="ExternalInput")
    ewA_d = nc.dram_tensor("ewA", [128, LA // 128], f32, kind="ExternalInput")
    idxB_d = nc.dram_tensor("idxB", [128, LB // 16], i16, kind="ExternalInput")
    ewB_d = nc.dram_tensor("ewB", [128, LB // 128], f32, kind="ExternalInput")
    rhoA_d = nc.dram_tensor("rhoA", [128, SLICE // 16], i16, kind="ExternalInput")
    rhoB_d = nc.dram_tensor("rhoB", [128, SLICE // 16], i16, kind="ExternalInput")
    W1_d = nc.dram_tensor("W1", [IN_DIM, HID], bf16, kind="ExternalInput")
    W2_d = nc.dram_tensor("W2", [HID, HID], bf16, kind="ExternalInput")
    W3_d = nc.dram_tensor("W3", [128, HID], bf16, kind="ExternalInput")  # rows 64-127 hold W3
    Wl12_d = nc.dram_tensor("Wl12", [128, OUT_DIM], bf16, kind="ExternalInput")
    Wl3_d = nc.dram_tensor("Wl3", [HID, OUT_DIM], bf16, kind="ExternalInput")
    bias_d = nc.dram_tensor("bias", [128, 3 * HID], f32, kind="ExternalInput")
    blin_d = nc.dram_tensor("blin", [128, OUT_DIM], f32, kind="ExternalInput")
    out_d = nc.dram_tensor("out", [128, SLOTS, OUT_DIM], f32, kind="ExternalOutput")

    # internal DRAM: per-wave slice staging + region tables + bounces
    slice_w = [nc.dram_tensor(f"slice_w{k}", [128, hi - lo, HID], f32)
               for k, (lo, hi) in enumerate(WAVES)]
    table1_d = nc.dram_tensor("table1", [R1_ROWS, HID], f32, addr_space="Shared")
    table23_d = nc.dram_tensor("table23", [R23_ROWS, HID], f32, addr_space="Shared")
    bounceA_d = nc.dram_tensor("bounceA", [SLICE, HID], f32)
    bounceB_d = nc.dram_tensor("bounceB", [SLICE, HID], f32)

    qctr = [0]

    def nextq():
        q = qctr[0] % NQ
        qctr[0] += 1
        return q

    with tile.TileContext(nc) as tc:
        with (
            tc.tile_pool(name="const", bufs=1) as constp,
            tc.tile_pool(name="acc", bufs=1) as accp,
            tc.tile_pool(name="ht", bufs=1) as htp,
            tc.tile_pool(name="stag", bufs=1) as stagp,
            tc.tile_pool(name="msg", bufs=6) as msgp,
            tc.tile_pool(name="ps", bufs=3, space="PSUM") as psp,
            tc.tile_pool(name="pso", bufs=2, space="PSUM") as psop,
        ):
            # ---- load constants ----
            xT = constp.tile([IN_DIM, SLICE], bf16)
            idxA = constp.tile([128, LA // 16], i16)
            ewA = constp.tile([128, LA // 128], f32)
            idxB = constp.tile([128, LB // 16], i16)
            ewB = constp.tile([128, LB // 128], f32)
            rhoA = constp.tile([128, SLICE // 16], i16)
            rhoB = constp.tile([128, SLICE // 16], i16)
            W1t = constp.tile([IN_DIM, HID], bf16)
            W2t = constp.tile([HID, HID], bf16)
            W3t = constp.tile([128, HID], bf16)
            Wl12t = constp.tile([128, OUT_DIM], bf16)
            Wl3t = constp.tile([HID, OUT_DIM], bf16)
            biast = constp.tile([128, 3 * HID], f32)
            blint = constp.tile([128, OUT_DIM], f32)
            ident = constp.tile([128, 128], bf16)

            for t, d in ((xT, xT_d), (idxA, idxA_d), (ewA, ewA_d),
                         (idxB, idxB_d), (ewB, ewB_d), (rhoA, rhoA_d),
                         (rhoB, rhoB_d), (W1t, W1_d), (W2t, W2_d),
                         (Wl12t, Wl12_d), (Wl3t, Wl3_d), (biast, bias_d),
                         (blint, blin_d)):
                nc.sync.dma_start(t[:], d[:])
            nc.sync.dma_start(W3t[:], W3_d[:])
            from concourse.masks import make_identity
            make_identity(nc, ident[:])

            h12T = htp.tile([128, SLICE], bf16)   # rows 0-63: h1^T, 64-127: h2^T
            h3T = htp.tile([HID, SLICE], bf16)

            relu = mybir.ActivationFunctionType.Relu

            def emit_ag(k):
                lo, hi = WAVES[k]
                if k == 0:
                    out_ap = table1_d[:]
                elif k == 1:
                    out_ap = table23_d[0:16384, :]
                else:
                    out_ap = table23_d[16384:32768, :]
                nc.gpsimd.collective_compute(
                    "AllGather", mybir.AluOpType.bypass,
                    replica_groups=[list(range(M))],
                    ins=[slice_w[k][:]], outs=[out_ap],
                )

            # ---- layer-1 input matmuls, suffix-first, AG per wave ----
            stag = stagp.tile([128, SLOTS, HID], f32, tag="stag")
            for k, (lo, hi) in enumerate(WAVES):
                for m in range(lo, hi):
                    ps = psp.tile([128, HID], f32, tag="mm")
                    nc.tensor.matmul(ps[:], xT[:, m * 128:(m + 1) * 128], W1t[:],
                                     start=True, stop=True)
                    nc.vector.tensor_copy(stag[:, m, :], ps[:])
                nc.sync.dma_start(slice_w[k][:], stag[:, lo:hi, :])
                emit_ag(k)

            ostag = stagp.tile([128, SLOTS, OUT_DIM], f32, tag="ostag")

            for layer in range(3):
                accA = accp.tile([128, SLOTS, HID], f32, tag="accA")
                accB = accp.tile([128, SLOTS, HID], f32, tag="accB")
                accT = accp.tile([128, SLOTS, HID], f32, tag="accT")
                acc_bf = accp.tile([128, SLOTS, HID], bf16, tag="accbf")
                nc.vector.memset(accA[:], 0.0)
                nc.vector.memset(accB[:], 0.0)

                # tiny warmup gathers on each queue so post-collective DGE
                # state reload happens off the critical path (discarded)
                for _ in range(NQ):
                    wmsg = msgp.tile([128, CMAX // 128, HID], f32, tag="msg")
                    nc.gpsimd.dma_gather(
                        wmsg[:, :1, :], bounceB_d[:], rhoB[:, 0:8],
                        128, 128, HID, single_packet=False, queue_num=nextq())

                def emit_chunks(acc, idx_t, ew_t, chunks, tbl):
                    for (off, w, segs) in chunks:
                        ws = w // 128
                        msg = msgp.tile([128, CMAX // 128, HID], f32, tag="msg")
                        nc.gpsimd.dma_gather(
                            msg[:, :ws, :], tbl, idx_t[:, off // 16:(off + w) // 16],
                            w, w, HID, single_packet=False, queue_num=nextq())
                        nc.vector.tensor_mul(
                            msg[:, :ws, :], msg[:, :ws, :],
                            ew_t[:, off // 128:(off + w) // 128]
                            .to_broadcast([128, ws, HID]))
                        for (ms, as_, ns) in segs:
                            nc.vector.tensor_add(
                                acc[:, as_:as_ + ns, :], acc[:, as_:as_ + ns, :],
                                msg[:, ms:ms + ns, :])

                def emit_fold(k, bounce_d, rho_t, first):
                    lo, hi = WAVES[k]
                    w = (hi - lo) * 128
                    ws = hi - lo
                    off = lo * 128
                    msg = msgp.tile([128, CMAX // 128, HID], f32, tag="msg")
                    nc.gpsimd.dma_gather(
                        msg[:, :ws, :], bounce_d[:],
                        rho_t[:, off // 16:(off + w) // 16],
                        w, w, HID, single_packet=False, queue_num=nextq())
                    if first:
                        nc.vector.tensor_copy(accT[:, lo:hi, :], msg[:, :ws, :])
                    else:
                        nc.vector.tensor_add(accT[:, lo:hi, :], accT[:, lo:hi, :],
                                             msg[:, :ws, :])

                def emit_tail(k):
                    lo, hi = WAVES[k]
                    nsl = hi - lo
                    bslice = biast[:, layer * HID:(layer + 1) * HID]
                    nc.vector.tensor_add(
                        accT[:, lo:hi, :], accT[:, lo:hi, :],
                        bslice.rearrange("p (s d) -> p s d", s=1)
                        .to_broadcast([128, nsl, HID]))
                    nc.scalar.activation(acc_bf[:, lo:hi, :], accT[:, lo:hi, :],
                                         relu)
                    for m in range(lo, hi):
                        pst = psp.tile([HID, 128], bf16, tag="tr")
                        nc.tensor.transpose(pst[:], acc_bf[:, m, :], ident[:])
                        sl = slice(m * 128, (m + 1) * 128)
                        if layer == 0:
                            nc.vector.tensor_copy(h12T[0:HID, sl], pst[:])
                        elif layer == 1:
                            nc.vector.tensor_copy(h12T[HID:128, sl], pst[:])
                        else:
                            nc.vector.tensor_copy(h3T[:, sl], pst[:])
                    for m in range(lo, hi):
                        sl = slice(m * 128, (m + 1) * 128)
                        if layer == 0:
                            ps = psp.tile([128, HID], f32, tag="mm")
                            nc.tensor.matmul(ps[:], h12T[0:HID, sl], W2t[:],
                                             start=True, stop=True)
                            nc.vector.tensor_copy(stag[:, m, :], ps[:])
                        elif layer == 1:
                            ps = psp.tile([128, HID], f32, tag="mm")
                            nc.tensor.matmul(ps[:], h12T[HID:128, sl],
                                             W3t[HID:128, :],
                                             start=True, stop=True)
                            nc.vector.tensor_copy(stag[:, m, :], ps[:])
                        else:
                            pso = psop.tile([128, OUT_DIM], f32, tag="out")
                            nc.tensor.matmul(pso[:], h12T[:, sl],
                                             Wl12t[:], start=True, stop=False)
                            nc.tensor.matmul(pso[:], h3T[:, sl],
                                             Wl3t[:], start=False, stop=True)
                            nc.vector.tensor_add(ostag[:, m, :], pso[:], blint[:])
                    if layer < 2:
                        nc.sync.dma_start(slice_w[k][:], stag[:, lo:hi, :])
                        emit_ag(k)
                    else:
                        nc.sync.dma_start(out_d[:, lo:hi, :], ostag[:, lo:hi, :])

                # sys B first (gated on AG-1 only), bounce it while sys A runs
                emit_chunks(accB, idxB, ewB, plan["chunksB"], table1_d[:])
                nc.sync.dma_start(
                    bounceB_d[:].rearrange("(p s) d -> p s d", p=128), accB[:])
                if layer < 2:
                    stag = stagp.tile([128, SLOTS, HID], f32, tag="stag")

                chA = plan["chunksA"]
                emit_chunks(accA, idxA, ewA, chA[:2], table23_d[:])
                for k in range(3):
                    emit_fold(k, bounceB_d, rhoB, first=True)
                emit_chunks(accA, idxA, ewA, chA[2:], table23_d[:])
                nc.sync.dma_start(
                    bounceA_d[:].rearrange("(p s) d -> p s d", p=128), accA[:])

                for k in range(3):
                    emit_fold(k, bounceA_d, rhoA, first=False)
                    emit_tail(k)

    nc.compile()
    return nc


_CACHE = {}


def kernel(x, edge_index, edge_weight, W1, b1, W2, b2, W3, b3, Wlin, blin):
    from concourse.bass_utils import run_bass_kernel_spmd

    x = np.asarray(x, dtype=np.float32)
    assert x.shape == (N, IN_DIM) and np.asarray(edge_index).shape == (2, E)

    key = hash(np.asarray(edge_index).tobytes())
    if key not in _CACHE:
        plan, in_maps = _prep(x, edge_index, edge_weight)
        nc = _build(plan, W1, b1, W2, b2, W3, b3, Wlin, blin)
        _CACHE[key] = (plan, nc)
    else:
        plan, nc = _CACHE[key]
        _, in_maps = _prep(x, edge_index, edge_weight)

    import ml_dtypes
    bf16 = ml_dtypes.bfloat16
    Wlin = np.asarray(Wlin, dtype=np.float32)
    shared = {
        "W1": np.asarray(W1, np.float32).astype(bf16),
        "W2": np.asarray(W2, np.float32).astype(bf16),
        "W3": np.concatenate([np.zeros((HID, HID), np.float32), np.asarray(W3, np.float32)], axis=0).astype(bf16),
        "Wl12": np.ascontiguousarray(Wlin[0:128]).astype(bf16),
        "Wl3": np.ascontiguousarray(Wlin[128:192]).astype(bf16),
        "bias": np.tile(np.concatenate([np.asarray(b, np.float32) for b in (b1, b2, b3)])[None, :], (128, 1)),
        "blin": np.tile(np.asarray(blin, np.float32)[None, :], (128, 1)),
    }
    for im in in_maps:
        im["xT"] = im["xT"].astype(bf16)
        im.update(shared)

    res = run_bass_kernel_spmd(nc, in_maps, core_ids=list(range(M)))
    kernel._last_results = res
    kernel._last_in_maps = in_maps
    kernel._last_nc = nc

    out = np.empty((N, OUT_DIM), np.float32)
    q = np.arange(NPC)
    for c in range(M):
        oc = res.results[c]["out"]  # [128, SLOTS, OUT]
        out[c * NPC:(c + 1) * NPC] = oc[q % 128, q // 128, :]
    return out
